# revision 1
# baseline (speedup 1.0000x reference)
"""Trainium2 Bass kernel for a pre-LN transformer block (B=2,T=2048,C=768,H=12,F=3072).

Sharding: pure data-parallel over 8 cores = 2 batches x 4 query-groups. Every
core runs an identical SPMD program; per-core differences are carried by data:
the host sends each core a row-PERMUTED copy of its batch's x so that the
core's own query tiles sit at fixed positions (first 128 rows of each 512-row
chunk), and causality is applied via a per-core mask tensor (attention is
permutation-invariant over keys).

Layouts: activations row-major for LN/residuals, feature-major (via PE
transpose) for matmul contraction. Attention uses an S^T (key-major) sweep: no
softmax max-subtraction (|S| < 1 here), denominator via a ones-column appended
to v, normalization deferred to the [64,512] per-head output. Matmuls run
fp32r (full-rate fp32); q/k/v/E in bf16. Weights are host-pre-tiled so each
load is one large contiguous DMA (HWDGE per-instruction overhead dominates
small DMAs).
"""
import sys

sys.path.insert(0, "/opt/trn_rl_repo")
sys.path.insert(0, "/opt/trn_rl_repo/concourse")

from contextlib import ExitStack

import numpy as np

import concourse.bass as bass
import concourse.tile as tile
from concourse import bacc, mybir
from concourse.bass_utils import run_bass_kernel_spmd
from concourse.masks import make_identity

B, T, C, H, D, F = 2, 2048, 768, 12, 64, 3072
EPS = 1e-5
NCORES = 8
QUAD = 4          # cores per batch
NJ = 4            # q-tiles of 128 per core
R = 512           # rows per core
NRT = T // 128    # 16 row tiles of x_full
NCB = C // 128    # 6 feature chunks
NFT = F // 128    # 24 mlp feature chunks

F32 = mybir.dt.float32
F32R = mybir.dt.float32r
BF16 = mybir.dt.bfloat16


def build_program(with_cv=True):
    nc = bacc.Bacc("TRN2", target_bir_lowering=False, debug=False,
                   num_devices=NCORES)
    # ---- DRAM I/O ----
    x_full = nc.dram_tensor("x_full", (T, C), F32, kind="ExternalInput").ap()
    msk_d = nc.dram_tensor("msk", (128, 512), BF16, kind="ExternalInput").ap()
    wq_d = nc.dram_tensor("wq", (NCB * NCB * 128, 128), F32R, kind="ExternalInput").ap()
    wk_d = nc.dram_tensor("wk", (NCB * NCB * 128, 128), F32R, kind="ExternalInput").ap()
    wv_d = nc.dram_tensor("wv", (C, C), F32R, kind="ExternalInput").ap()
    wp_d = nc.dram_tensor("wp", (C, C), F32R, kind="ExternalInput").ap()
    cqk_d = nc.dram_tensor("cqk", (128, 12), F32, kind="ExternalInput").ap()
    cv_d = nc.dram_tensor("cv", (1, C), F32R, kind="ExternalInput").ap()
    bp_d = nc.dram_tensor("bp", (1, C), F32R, kind="ExternalInput").ap()
    w1_d = nc.dram_tensor("w1", (NFT * NCB * 128, 128), F32R, kind="ExternalInput").ap()
    c1_d = nc.dram_tensor("c1", (128, NFT), F32, kind="ExternalInput").ap()
    w2_d = nc.dram_tensor("w2", (NCB * NFT * 128, 128), F32R, kind="ExternalInput").ap()
    b2c_d = nc.dram_tensor("b2c", (128, NCB), F32, kind="ExternalInput").ap()
    ones_d = nc.dram_tensor("ones1", (1, 512), F32R, kind="ExternalInput").ap()
    out_d = nc.dram_tensor("out", (R, C), F32, kind="ExternalOutput").ap()

    Exp = mybir.ActivationFunctionType.Exp
    Relu = mybir.ActivationFunctionType.Relu
    Ident = mybir.ActivationFunctionType.Identity
    Sqrt = mybir.ActivationFunctionType.Sqrt
    MUL = mybir.AluOpType.mult
    ADD = mybir.AluOpType.add
    SUB = mybir.AluOpType.subtract

    with tile.TileContext(nc) as tc, ExitStack() as top:
        const = top.enter_context(tc.tile_pool(name="const", bufs=1))
        ident = const.tile([128, 128], F32)
        make_identity(nc, ident[:])
        epsc = const.tile([128, 1], F32)
        nc.vector.memset(epsc[:], EPS)
        ones = const.tile([1, 512], F32R)
        msk = const.tile([128, 512], BF16)
        cqk = const.tile([128, 12], F32)
        cv = const.tile([1, C], F32R)
        bp = const.tile([1, C], F32R)
        c1 = const.tile([128, NFT], F32)
        b2c = const.tile([128, NCB], F32)

        def load_consts():
            nc.sync.dma_start(ones[:], ones_d)
            nc.sync.dma_start(msk[:], msk_d)
            nc.sync.dma_start(cqk[:], cqk_d)
            nc.sync.dma_start(cv[:], cv_d)
            nc.sync.dma_start(bp[:], bp_d)
            nc.sync.dma_start(c1[:], c1_d)
            nc.sync.dma_start(b2c[:], b2c_d)

        # persistent tiles
        act = top.enter_context(tc.tile_pool(name="act", bufs=1))
        xo_sb = [act.tile([128, C], F32, tag=f"xo{j}", name=f"xo{j}") for j in range(NJ)]
        x2 = [act.tile([128, C], F32, tag=f"x2{j}", name=f"x2{j}") for j in range(NJ)]

        z2pool = top.enter_context(tc.tile_pool(name="z2p", bufs=1))
        z2fm = [z2pool.tile([128, R], F32R, tag=f"z2{cb}", name=f"z2{cb}")
                for cb in range(NCB)]

        stats = top.enter_context(tc.tile_pool(name="stats", bufs=3))

        # attention-lifetime tensors (released after proj)
        kvat = tc.alloc_tile_pool(name="kvat", bufs=1)
        qfm = [kvat.tile([128, R], BF16, tag=f"qf{ct}", name=f"qf{ct}")
               for ct in range(NCB)]
        kfm = [[kvat.tile([128, 512], BF16, tag=f"kf{ct}_{rc}", name=f"kf{ct}_{rc}")
                for rc in range(4)] for ct in range(NCB)]
        vrm = [kvat.tile([128, H * 65], BF16, tag=f"vr{rt}", name=f"vr{rt}")
               for rt in range(NRT)]
        afm = [kvat.tile([128, R], F32R, tag=f"af{cb}", name=f"af{cb}")
               for cb in range(NCB)]

        def ln_tile(x_ap):
            st = stats.tile([128, 12], F32, tag="lnst")
            nc.vector.bn_stats(st[:, 0:6], x_ap[:, 0:384])
            nc.vector.bn_stats(st[:, 6:12], x_ap[:, 384:768])
            mv = stats.tile([128, 2], F32, tag="lnmv")
            nc.vector.bn_aggr(mv[:], st[:].rearrange("p (g k) -> p g k", g=2))
            sd = stats.tile([128, 1], F32, tag="lnsd")
            nc.scalar.activation(sd[:], mv[:, 1:2], Sqrt, bias=epsc[:])
            rr = stats.tile([128, 1], F32, tag="lnrr")
            nc.vector.reciprocal(rr[:], sd[:])
            zt = stats.tile([128, C], F32, tag="lnz", bufs=2)
            nc.vector.tensor_scalar(zt[:], x_ap, mv[:, 0:1], rr[:],
                                    op0=SUB, op1=MUL)
            return zt

        with ExitStack() as phase1:
            zpool = phase1.enter_context(tc.tile_pool(name="zfm", bufs=1))
            zfm = [zpool.tile([128, T], F32R, tag=f"z{cb}", name=f"z{cb}")
                   for cb in range(NCB)]
            ld = phase1.enter_context(tc.tile_pool(name="ld", bufs=2))
            wst = phase1.enter_context(tc.tile_pool(name="wst", bufs=2))
            wvh = phase1.enter_context(tc.tile_pool(name="wvh", bufs=1))
            tp = phase1.enter_context(tc.tile_pool(name="tp", bufs=2, space="PSUM"))
            kqp = phase1.enter_context(tc.tile_pool(name="kqp", bufs=2, space="PSUM"))
            vp_ = phase1.enter_context(tc.tile_pool(name="vp", bufs=2, space="PSUM"))

            def emit_q():
                for ct in range(NCB):
                    wqg = wst.tile([128, C], F32R, tag="wq", name=f"wqg{ct}", bufs=2)
                    nc.sync.dma_start(wqg[:].rearrange("p (cb c) -> p cb c", cb=NCB),
                                      wq_d[768 * ct: 768 * ct + 768, :]
                                      .rearrange("(cb p) c -> p cb c", p=128))
                    qp = kqp.tile([128, 512], F32, tag="qp")
                    for cb in range(NCB):
                        nc.tensor.matmul(
                            qp[:].rearrange("p (j u) -> p j u", j=NJ),
                            wqg[:, 128 * cb:128 * cb + 128],
                            zfm[cb][:].rearrange("p (j u) -> p j u", u=512)[:, :, 0:128],
                            start=(cb == 0), stop=(cb == NCB - 1))
                    nc.scalar.activation(qfm[ct][:], qp[:], Ident,
                                         bias=cqk[:, ct:ct + 1])

            # ---- Stage A: x load + LN1 + transpose -> zfm; xo_sb slices ----
            load_consts()
            rt_order = [0, 4, 8, 12] + [rt for rt in range(NRT) if rt % 4 != 0]
            for rt in rt_order:
                xh = ld.tile([128, C], F32, tag="xf", name=f"xh{rt}", bufs=2)
                nc.sync.dma_start(xh[:], x_full[128 * rt:128 * rt + 128, :])
                if rt == 1:
                    emit_q()
                if True:
                    zt = ln_tile(xh[:])
                    for cp in range(NCB // 2):
                        pt = tp.tile([128, 256], F32, tag="zt")
                        for u in range(2):
                            cb = 2 * cp + u
                            nc.tensor.transpose(pt[:, 128 * u:128 * u + 128],
                                                zt[:, 128 * cb:128 * cb + 128],
                                                ident[:])
                        for u in range(2):
                            cb = 2 * cp + u
                            nc.any.tensor_copy(
                                zfm[cb][:, 128 * rt:128 * rt + 128],
                                pt[:, 128 * u:128 * u + 128])

            # ---- Stage B: k (fm), q (fm), v (rm + ones cols) ----
            for j in range(NJ):
                nc.sync.dma_start(xo_sb[j][:], x_full[512 * j:512 * j + 128, :])
            for ct in range(NCB):
                wkg = wst.tile([128, C], F32R, tag="wk", name=f"wkg{ct}", bufs=2)
                nc.sync.dma_start(wkg[:].rearrange("p (cb c) -> p cb c", cb=NCB),
                                  wk_d[768 * ct: 768 * ct + 768, :]
                                  .rearrange("(cb p) c -> p cb c", p=128))
                for rc in range(4):
                    kp = kqp.tile([128, 512], F32, tag="kp")
                    for cb in range(NCB):
                        nc.tensor.matmul(kp[:], wkg[:, 128 * cb:128 * cb + 128],
                                         zfm[cb][:, 512 * rc:512 * rc + 512],
                                         start=(cb == 0), stop=(cb == NCB - 1))
                    nc.scalar.activation(kfm[ct][rc][:], kp[:], Ident,
                                         bias=cqk[:, 6 + ct:7 + ct])
            for rt in range(NRT):
                nc.vector.memset(
                    vrm[rt][:].rearrange("p (n k) -> p n k", k=65)[:, :, 64:65], 1.0)
            for hf in range(2):
                wv_t = []
                for cb in range(NCB):
                    w = wvh.tile([128, 384], F32R, tag=f"wv{cb}", name=f"wv{cb}_{hf}")
                    nc.sync.dma_start(w[:], wv_d[128 * cb:128 * cb + 128,
                                                 384 * hf:384 * hf + 384])
                    wv_t.append(w)
                for rt in range(NRT):
                    vp = vp_.tile([128, 384], F32, tag="vp")
                    for cb in range(NCB):
                        nc.tensor.matmul(vp[:],
                                         zfm[cb][:, 128 * rt:128 * rt + 128],
                                         wv_t[cb][:],
                                         start=(cb == 0),
                                         stop=(not with_cv and cb == NCB - 1),
                                         skip_group_check=True)
                    if with_cv:
                        nc.tensor.matmul(vp[:], ones[0:1, 0:128],
                                         cv[0:1, 384 * hf:384 * hf + 384],
                                         start=False, stop=True,
                                         skip_group_check=True)
                    dst = vrm[rt][:, 65 * 6 * hf: 65 * 6 * hf + 65 * 6]
                    nc.any.tensor_copy(
                        dst.rearrange("p (h k) -> p h k", k=65)[:, :, 0:64],
                        vp[:].rearrange("p (h k) -> p h k", k=64))

        # ---- prefetch proj weights during attention (DMA idle there) ----
        wpp = tc.alloc_tile_pool(name="wpp", bufs=1)
        wp_t = []
        for cb in range(NCB):
            w = wpp.tile([128, C], F32R, tag=f"wp{cb}", name=f"wp{cb}")
            nc.sync.dma_start(w[:], wp_d[128 * cb:128 * cb + 128, :])
            wp_t.append(w)

        # ---- Stage C: attention (S^T sweep, kb pairs merged) ----
        with ExitStack() as phase2:
            ep = phase2.enter_context(tc.tile_pool(name="ep", bufs=2))
            sp_ = phase2.enter_context(tc.tile_pool(name="sp", bufs=3, space="PSUM"))
            app = phase2.enter_context(tc.tile_pool(name="app", bufs=1, space="PSUM"))
            bcp = phase2.enter_context(tc.tile_pool(name="bcp", bufs=1, space="PSUM"))
            for h in range(H):
                hb, ho = h // 2, 64 * (h % 2)
                ap = app.tile([128, 512], F32, tag="ap")
                es = []
                for c in range(4):
                    n = 512 - 128 * c
                    for pr in range(2):
                        sp = sp_.tile([128, 1024], F32, tag="sp")
                        for hf in range(2):
                            kb = 2 * pr + hf
                            nc.tensor.matmul(
                                sp[:, 512 * hf:512 * hf + n],
                                kfm[hb][c][ho:ho + 64, 128 * kb:128 * kb + 128],
                                qfm[hb][ho:ho + 64, 128 * c: 512],
                                start=True, stop=True)
                        e = ep.tile([128, 1024], BF16, tag="e", bufs=16)
                        nc.scalar.activation(
                            e[:].rearrange("p (b n) -> p b n", b=2)[:, :, 0:n],
                            sp[:].rearrange("p (b n) -> p b n", b=2)[:, :, 0:n], Exp)
                        nc.vector.tensor_tensor(
                            e[:].rearrange("p (b n) -> p b n", b=2)[:, :, 0:128],
                            e[:].rearrange("p (b n) -> p b n", b=2)[:, :, 0:128],
                            msk[:, 256 * pr:256 * pr + 256]
                                .rearrange("p (b n) -> p b n", b=2), op=MUL)
                        es.append((c, n, pr, e))
                for c, n, pr, e in es:
                    for hf in range(2):
                        kb = 2 * pr + hf
                        nc.tensor.matmul(ap[0:65, 128 * c:512],
                                         vrm[4 * c + kb][:, 65 * h:65 * h + 65],
                                         e[:, 512 * hf:512 * hf + n],
                                         start=(c == 0 and kb == 0),
                                         stop=(c == 3 and kb == 3),
                                         skip_group_check=True)
                invd = ep.tile([1, 512], F32R, tag="invd")
                with nc.allow_low_precision(reason="fp32r invd for broadcast mm"):
                    nc.vector.reciprocal(invd[:], ap[64:65, :])
                bc = bcp.tile([128, 512], F32, tag="bc")
                nc.tensor.matmul(bc[0:64, :], ones[0:1, 0:64], invd[:],
                                 start=True, stop=True)
                raw = ep.tile([64, 512], F32, tag="raw")
                nc.vector.tensor_copy(raw[:], ap[0:64, :])
                nc.vector.tensor_tensor(afm[hb][ho:ho + 64, :],
                                        raw[:], bc[0:64, :], op=MUL)

        # ---- Stage D: proj + residual + LN2 ----
        with ExitStack() as phase3:
            pp_ = phase3.enter_context(tc.tile_pool(name="pp", bufs=2, space="PSUM"))
            tp2 = phase3.enter_context(tc.tile_pool(name="tp2", bufs=2, space="PSUM"))
            for j in range(NJ):
                pp = pp_.tile([128, C], F32, tag="pp")
                for no, nn in ((0, 512), (512, 256)):
                    for cb in range(NCB):
                        nc.tensor.matmul(pp[:, no:no + nn],
                                         afm[cb][:, 128 * j: 128 * j + 128],
                                         wp_t[cb][:, no:no + nn],
                                         start=(cb == 0), stop=False,
                                         skip_group_check=True)
                    nc.tensor.matmul(pp[:, no:no + nn], ones[0:1, 0:128],
                                     bp[0:1, no:no + nn], start=False, stop=True,
                                     skip_group_check=True)
                nc.vector.tensor_tensor(x2[j][:], xo_sb[j][:], pp[:], op=ADD)
                zt = ln_tile(x2[j][:])
                for cb in range(NCB):
                    pt = tp2.tile([128, 128], F32, tag="zt2")
                    nc.tensor.transpose(pt[:], zt[:, 128 * cb:128 * cb + 128], ident[:])
                    nc.any.tensor_copy(z2fm[cb][:, 128 * j: 128 * j + 128], pt[:])
        wpp.release()
        kvat.release()

        # ---- Stage F/G: MLP ----
        outp = tc.alloc_tile_pool(name="outp", bufs=1)
        out_sb = [outp.tile([128, C], F32, tag=f"ou{j}", name=f"ou{j}")
                  for j in range(NJ)]
        with ExitStack() as phase4:
            a1pool = phase4.enter_context(tc.tile_pool(name="a1", bufs=1))
            a1 = [a1pool.tile([128, R], F32R, tag=f"a1{ft}", name=f"a1{ft}")
                  for ft in range(NFT)]
            w1st = phase4.enter_context(tc.tile_pool(name="w1st", bufs=8))
            w2st = phase4.enter_context(tc.tile_pool(name="w2st", bufs=4))
            mp_ = phase4.enter_context(tc.tile_pool(name="mp", bufs=3, space="PSUM"))
            fp_ = phase4.enter_context(tc.tile_pool(name="fp", bufs=3, space="PSUM"))
            ftp = phase4.enter_context(tc.tile_pool(name="ftp", bufs=2, space="PSUM"))
            ffs_ = phase4.enter_context(tc.tile_pool(name="ffs", bufs=2))
            for ft in range(NFT):
                w1g = w1st.tile([128, C], F32R, tag="w1", name=f"w1g{ft}")
                nc.sync.dma_start(w1g[:].rearrange("p (cb c) -> p cb c", cb=NCB),
                                  w1_d[768 * ft: 768 * ft + 768, :]
                                  .rearrange("(cb p) c -> p cb c", p=128))
                mp = mp_.tile([128, R], F32, tag="mp")
                for hv in range(2):
                    for cb in range(NCB):
                        nc.tensor.matmul(mp[:, 256 * hv:256 * hv + 256],
                                         w1g[:, 128 * cb:128 * cb + 128],
                                         z2fm[cb][:, 256 * hv:256 * hv + 256],
                                         start=(cb == 0), stop=(cb == NCB - 1),
                                         skip_group_check=True)
                nc.scalar.activation(a1[ft][:], mp[:], Relu, bias=c1[:, ft:ft + 1])
            for ct in range(NCB):
                w2g = []
                for hf in range(2):
                    wg = w2st.tile([128, 12 * 128], F32R, tag="w2",
                                   name=f"w2g{ct}_{hf}")
                    nc.sync.dma_start(
                        wg[:].rearrange("p (ft c) -> p ft c", ft=12),
                        w2_d[3072 * ct + 1536 * hf: 3072 * ct + 1536 * hf + 1536, :]
                        .rearrange("(ft p) c -> p ft c", p=128))
                    w2g.append(wg)
                fp = fp_.tile([128, R], F32, tag="fp")
                for ft in range(NFT):
                    nc.tensor.matmul(
                        fp[:],
                        w2g[ft // 12][:, 128 * (ft % 12): 128 * (ft % 12) + 128],
                        a1[ft][:],
                        start=(ft == 0), stop=(ft == NFT - 1))
                ffs = ffs_.tile([128, R], F32, tag="ffs")
                nc.scalar.activation(ffs[:], fp[:], Relu, bias=b2c[:, ct:ct + 1])
                for j in range(NJ):
                    pt = ftp.tile([128, 128], F32, tag="ftp")
                    nc.tensor.transpose(pt[:], ffs[:, 128 * j:128 * j + 128], ident[:])
                    nc.vector.tensor_tensor(
                        out_sb[j][:, 128 * ct: 128 * ct + 128],
                        x2[j][:, 128 * ct: 128 * ct + 128],
                        pt[:], op=ADD)

        for j in range(NJ):
            nc.sync.dma_start(out_d[128 * j:128 * j + 128, 0:384],
                              out_sb[j][:, 0:384])
        for j in range(NJ):
            nc.sync.dma_start(out_d[128 * j:128 * j + 128, 384:768],
                              out_sb[j][:, 384:768])
        outp.release()

    nc.finalize()
    return nc


_CACHE = {}


def _get_nc(with_cv=True):
    key = ("nc", with_cv)
    if key not in _CACHE:
        _CACHE[key] = build_program(with_cv=with_cv)
    return _CACHE[key]


def _host_prep(inputs):
    import ml_dtypes
    x = np.ascontiguousarray(np.asarray(inputs["x"], np.float32))
    Wq = np.asarray(inputs["Wq"], np.float32).transpose(1, 0, 2).reshape(C, C)
    Wk = np.asarray(inputs["Wk"], np.float32).transpose(1, 0, 2).reshape(C, C)
    Wv = np.asarray(inputs["Wv"], np.float32).transpose(1, 0, 2).reshape(C, C)
    g1 = np.asarray(inputs["ln1_g"], np.float32)
    b1l = np.asarray(inputs["ln1_b"], np.float32)
    g2 = np.asarray(inputs["ln2_g"], np.float32)
    b2l = np.asarray(inputs["ln2_b"], np.float32)
    s = np.float32(C ** -0.5)

    def tile_cm(w, ncol):
        nr = w.shape[0] // 128
        return np.ascontiguousarray(
            w.reshape(nr, 128, ncol, 128).transpose(2, 0, 1, 3)
            .reshape(ncol * nr * 128, 128))

    wq = tile_cm(np.ascontiguousarray(g1[:, None] * Wq * s), NCB)
    wk = tile_cm(np.ascontiguousarray(g1[:, None] * Wk), NCB)
    wv = np.ascontiguousarray(g1[:, None] * Wv)
    cq = (b1l @ Wq) * s
    ck = b1l @ Wk
    cv = np.ascontiguousarray((b1l @ Wv).reshape(1, C))
    cqk = np.ascontiguousarray(
        np.concatenate([cq.reshape(NCB, 128).T, ck.reshape(NCB, 128).T], axis=1))
    W1 = np.asarray(inputs["W1"], np.float32)
    w1 = tile_cm(np.ascontiguousarray(g2[:, None] * W1), NFT)
    c1 = np.ascontiguousarray((b2l @ W1 + np.asarray(inputs["b1"], np.float32))
                              .reshape(NFT, 128).T)
    wp = np.ascontiguousarray(np.asarray(inputs["Wp"], np.float32))
    bp = np.ascontiguousarray(np.asarray(inputs["bp"], np.float32).reshape(1, C))
    w2 = tile_cm(np.asarray(inputs["W2"], np.float32), NCB)
    b2c = np.ascontiguousarray(
        np.asarray(inputs["b2"], np.float32).reshape(NCB, 128).T)

    in_maps = []
    row_idx = []
    i128 = np.arange(128)
    for core in range(NCORES):
        b, g = core // QUAD, core % QUAD
        # permutation: position 512c + 128u + i -> global row 128((g+u)%4 + 4c) + i
        perm = np.concatenate(
            [128 * (((g + u) % 4) + 4 * c) + i128
             for c in range(4) for u in range(4)])
        own = np.concatenate([np.arange(128 * (g + 4 * j), 128 * (g + 4 * j) + 128)
                              for j in range(NJ)])
        row_idx.append((b, own))
        kl = i128[:, None]
        ql = i128[None, :]
        msk = np.zeros((128, 512), np.float32)
        for u in range(4):
            msk[:, 128 * u:128 * u + 128] = \
                (128 * ((g + u) % 4) + kl <= 128 * g + ql)
        in_maps.append({
            "x_full": np.ascontiguousarray(x[b][perm]),
            "msk": msk.astype(ml_dtypes.bfloat16),
            "wq": wq, "wk": wk, "wv": wv, "wp": wp,
            "cqk": cqk, "cv": cv, "bp": bp,
            "w1": w1, "c1": c1, "w2": w2, "b2c": b2c,
            "ones1": np.ones((1, 512), np.float32),
        })
    return in_maps, row_idx


def _run(inputs, trace=False):
    with_cv = bool(np.any(np.asarray(inputs["ln1_b"], np.float32) != 0))
    nc = _get_nc(with_cv=with_cv)
    in_maps, row_idx = _host_prep(inputs)
    res = run_bass_kernel_spmd(nc, in_maps, core_ids=list(range(NCORES)),
                               trace=trace)
    out = np.zeros((B, T, C), np.float32)
    for core in range(NCORES):
        b, rows = row_idx[core]
        out[b][rows] = res.results[core]["out"]
    return out, res


def kernel(**inputs):
    out, _ = _run(inputs, trace=False)
    return out

